# revision 25
# baseline (speedup 1.0000x reference)
"""Trainium2 kernel for the bilinear form y[b,k] = sum_ij x[b,i] x[b,j] W[i,j,k] + b[k].

Shapes: x (512, 784) f32, W (614656=784*784, 10) f32, b (10,) f32 -> y (512, 10) f32.

Strategy (8 NeuronCores):
  - Shard the j axis of W.reshape(784, 784, 10) across cores: 98 j's per core.
    Each core reads W/8 + full x (~2.5 MB in fp16); compute is the long pole.
  - Stage 1 (TensorE): U[b, (k,j)] = sum_i x[b,i] * W[i, j_shard, k], x^T tiles
    stationary, W shard moving, accumulating over 7 uniform 112-row i-tiles
    into 8 PSUM banks (4 batch tiles x 2 column halves = 5 k's x 98 j's).
  - Stage 2 (DVE): U * xs multiply (PSUM read) + reduce over j, ~1.14us per
    group; groups must close >=1.2us apart or the DVE backs up.
  - Host: y = sum_c y_part_c + b  (20 KB per core; no collectives needed).

Schedule (calibrated against HW traces):
  - First input chunk completes ~5.6us after kernel start (1.6us framework
    entry + 0.7us queue op + ~3.3us ring latency); total input delivery runs
    at ~0.22-0.3 MB/us; y DMA completion costs a fixed ~1.9us after issue;
    framework teardown is a fixed ~8.5us.  The PE runs at half clock until
    the HAM boost, ~6.3us of GAP-FREE PE activity after the first matmul;
    any PE idle gap resets the ramp (costing ~5us) -- so dummy warmup
    matmuls bridge from t~2.1 until real data lands.
  - xT is laid out bt-major so each batch-tile's stationary set (0.2 MB)
    arrives independently: the first group closes at ~9.5us instead of ~14.
  - Matmul order chases delivery group-major ((bt,h) columns of work), so
    the 8 PSUM groups close every ~1.4us and all but the last DVE stage-2
    hide under the matmul stream.
  - y leaves in 4 group-pair DMAs on the gpsimd ring as pairs complete.
"""

import numpy as np

D = 784
B = 512
C = 10
NCORES = 8
JS = D // NCORES  # 98 j's per core
JK = JS * C  # 980 free columns per core, laid out as (k, j)
HALF = JK // 2  # 490 = 5 k's x 98 j's -> one PSUM bank
KH = C // 2  # 5 k's per half
P = 128
B_TILES = B // P  # 4
IT = 7  # i-tiles
IP = D // IT  # 112 rows per i-tile (uniform, no padding)
N_WARMUP_MM = 12  # dummy matmuls bridging until the first chunks land

MM_DTYPE = "float16"  # dtype of the matmul operands (and their DMA)

_nc_cache = {}


def _build_nc():
    import concourse.bacc as bacc
    import concourse.mybir as mybir
    import concourse.tile as tile

    mm_dt = getattr(mybir.dt, MM_DTYPE)
    f32 = mybir.dt.float32

    nc = bacc.Bacc("TRN2", target_bir_lowering=False)

    # Partition-major DRAM layouts (see _make_in_maps).
    xT = nc.dram_tensor("xT", [IP, B_TILES, IT, P], mm_dt, kind="ExternalInput")
    xT2 = nc.dram_tensor("xT2", [IP, 2, B], mm_dt, kind="ExternalInput")
    w = nc.dram_tensor("w", [IP, 2, IT, HALF], mm_dt, kind="ExternalInput")
    xs = nc.dram_tensor("xs", [P, B_TILES, JS], mm_dt, kind="ExternalInput")
    y = nc.dram_tensor("y", [P, 2, B_TILES, KH], f32, kind="ExternalOutput")

    with tile.TileContext(nc) as tc:
        with (
            tc.tile_pool(name="wpool", bufs=8) as wpool,
            tc.tile_pool(name="xpool", bufs=4) as xpool,
            tc.tile_pool(name="xspool", bufs=1) as xspool,
            tc.tile_pool(name="ypool", bufs=1) as ypool,
            tc.tile_pool(name="scratch", bufs=4) as spool,
            tc.tile_pool(name="psum", bufs=8, space="PSUM") as psum_pool,
        ):
            # Dummy warmup operands; memset on gpsimd right after entry so
            # the first warmup matmul (and the HAM ramp clock) starts ASAP.
            dmy_s = spool.tile([IP, P], mm_dt, name="dmy_s", tag="dmy_s", bufs=1)
            dmy_m = spool.tile([IP, HALF], mm_dt, name="dmy_m", tag="dmy_m", bufs=1)
            nc.gpsimd.memset(dmy_s[:], 0.0)
            nc.gpsimd.memset(dmy_m[:], 0.0)

            w_sb = {}  # (it, h) -> [IP, HALF] view

            def w_dma(eng, h, c0, c1):
                wt = wpool.tile(
                    [IP, c1 - c0, HALF], mm_dt, name=f"w_h{h}c{c0}", tag=f"wh{h}"
                )
                eng.dma_start(wt[:], w[:, h, c0:c1, :])
                for it in range(c0, c1):
                    w_sb[(it, h)] = wt[:, it - c0, :]

            xT_sb = {}  # bt -> [IP, IT, P]

            def xt_dma(bt):
                xt = xpool.tile([IP, IT, P], mm_dt, name=f"xt_b{bt}", tag="xt")
                nc.scalar.dma_start(xt[:], xT[:, bt])
                xT_sb[bt] = xt

            # Issue order == per-ring delivery order (queue ops ~650ns each).
            # sync: wh0[0:2], wh0[2:4], wh0[4:7], wh1[0:2], wh1[4:7]
            # scalar: xt-bt0..3, wh1[2:4]
            # gpsimd: xs, then the y pairs (ring stays warm)
            w_dma(nc.sync, 0, 0, 2)
            xt2_sb = xpool.tile([IP, 2, B], mm_dt, name="xt2", tag="xt2", bufs=1)
            nc.scalar.dma_start(xt2_sb[:], xT2[:])
            xt_dma(0)
            xt_dma(1)
            w_dma(nc.sync, 0, 2, 4)
            w_dma(nc.sync, 0, 4, 6)
            w_dma(nc.sync, 0, 6, 7)
            xt_dma(2)
            xt_dma(3)
            w_dma(nc.sync, 1, 0, 2)
            w_dma(nc.scalar, 1, 2, 4)
            w_dma(nc.sync, 1, 4, 7)
            xs_sb = xspool.tile([P, B_TILES, JS], mm_dt)
            nc.gpsimd.dma_start(xs_sb[:], xs[:])

            # PSUM: 8 accumulation groups (bt, h), one bank each.
            pts = {}
            for h in range(2):
                for bt in range(B_TILES):
                    pts[(bt, h)] = psum_pool.tile(
                        [P, HALF], f32, name=f"pt_b{bt}h{h}", tag="pt", bufs=8
                    )

            def warmup(n, bank=(3, 1)):
                # dummy matmuls into a not-yet-opened group's bank; each one
                # opens and closes its own accumulation group.
                for _ in range(n):
                    nc.tensor.matmul(
                        pts[bank][:], dmy_s[:], dmy_m[:], start=True, stop=True
                    )

            # First warmups use only dmy_s (ready after its 242ns memset, vs
            # ~550ns for dmy_m): the PE (and the HAM ramp clock) starts ~0.5us
            # earlier.
            for _ in range(2):
                nc.tensor.matmul(
                    pts[(3, 1)][:, :P], dmy_s[:], dmy_s[:], start=True, stop=True
                )
            warmup(N_WARMUP_MM)

            y_t = ypool.tile([P, 2, B_TILES, KH], f32)

            def mm_pre(it, bt):
                # prefix matmuls: stationary from the duplicated it-major
                # xT2 slice (lands with the first W chunk), h0 groups only
                nc.tensor.matmul(
                    pts[(bt, 0)][:],
                    xt2_sb[:, it, bt * P : (bt + 1) * P],
                    w_sb[(it, 0)][:],
                    start=(it == 0),
                    stop=False,
                )

            def mm(it, bt, h):
                nc.tensor.matmul(
                    pts[(bt, h)][:],
                    xT_sb[bt][:, it, :],
                    w_sb[(it, h)][:],
                    start=(it == 0),
                    stop=(it == IT - 1),
                )

            def stage2(bt, h):
                # Multiply on DVE (PSUM read), then reduce over j on DVE.
                pt = pts[(bt, h)]
                scr = spool.tile(
                    [P, HALF], f32, name=f"scr{bt}{h}", tag="scr", bufs=2
                )
                s3 = scr[:].rearrange("p (kh j) -> p kh j", kh=KH)
                p3 = pt[:].rearrange("p (kh j) -> p kh j", kh=KH)
                xs3 = xs_sb[:, bt, None, :].broadcast_to([P, KH, JS])
                nc.vector.tensor_tensor(s3, p3, xs3, mybir.AluOpType.mult)
                nc.vector.tensor_reduce(
                    out=y_t[:, h, bt, :],
                    in_=s3,
                    op=mybir.AluOpType.add,
                    axis=mybir.AxisListType.X,
                )

            def y_dma(h, bt):
                # ship groups (bt-1, h) and (bt, h) together
                nc.gpsimd.dma_start(
                    y[:, h, bt - 1 : bt + 1, :], y_t[:, h, bt - 1 : bt + 1, :]
                )

            # Group-chasing stream: each step is a run of i-tiles for one
            # (bt, h) group, ordered so every run's W/xT chunks have landed
            # and the 8 closures spread ~1.4us apart.
            # "W" entries are single warmup fillers (into g31's still-unopened
            # bank) absorbing DMA arrival jitter without a PE gap/ramp reset.
            # Column splits for the last-closing groups: sub-group
            # accumulators live in a slice of the group's own bank plus banks
            # recycled from early-closed groups (pool rotation: ptR1->g00,
            # ptR2->g10, ptR3->g20 banks), so only a small stage-2 trails
            # the last matmul.
            SPL31 = [(0, 2), (2, 2), (4, 1)]
            SPL21 = [(0, 2), (2, 3)]
            ptX31 = {0: pts[(3, 1)]}
            ptX31[1] = psum_pool.tile([P, 2 * JS], f32, name="ptR1", tag="pt", bufs=8)
            ptX31[2] = psum_pool.tile([P, JS], f32, name="ptR2", tag="pt", bufs=8)
            ptX21 = {0: pts[(2, 1)]}
            ptX21[1] = psum_pool.tile([P, 3 * JS], f32, name="ptR3", tag="pt", bufs=8)

            def mm_sg(bt, it, sg, splits, ptx):
                k0, nk = splits[sg]
                out = ptx[sg][:, : nk * JS]
                wsl = w_sb[(it, 1)][:, k0 * JS : (k0 + nk) * JS]
                nc.tensor.matmul(
                    out, xT_sb[bt][:, it, :], wsl,
                    start=(it == 0), stop=(it == IT - 1),
                )

            def stage2_sg(bt, sg, splits, ptx):
                k0, nk = splits[sg]
                pt = ptx[sg][:, : nk * JS]
                scr = spool.tile(
                    [P, nk * JS], f32, name=f"scr{bt}s{sg}", tag="scr", bufs=2
                )
                s3 = scr[:].rearrange("p (kh j) -> p kh j", kh=nk)
                p3 = pt.rearrange("p (kh j) -> p kh j", kh=nk)
                xs3 = xs_sb[:, bt, None, :].broadcast_to([P, nk, JS])
                nc.vector.tensor_tensor(s3, p3, xs3, mybir.AluOpType.mult)
                nc.vector.tensor_reduce(
                    out=y_t[:, 1, bt, k0 : k0 + nk],
                    in_=s3,
                    op=mybir.AluOpType.add,
                    axis=mybir.AxisListType.X,
                )

            def mm31(it, sg):
                mm_sg(3, it, sg, SPL31, ptX31)

            def stage2_31(sg):
                stage2_sg(3, sg, SPL31, ptX31)

            SCHED = [
                ("P", 0), ("P", 1),           # its0-1 h0 x 4bt via xT2
                "W",
                (0, 0, 2, 4), "W",
                (0, 0, 4, 7),                 # g00 close c1
                (1, 0, 2, 7),                 # g10 close c2
                "W",
                (2, 0, 2, 7),                 # g20 close c3
                (3, 0, 2, 7),                 # g30 close c4
                (0, 1, 0, 7),                 # g01 close c5
                (1, 1, 0, 7),                 # g11 close c6
                (2, 1, 0, 7),                 # g21 split close
                (3, 1, 0, 7),                 # g31 split close
            ]
            g31_opened = False
            for step in SCHED:
                if step == "W":
                    if not g31_opened:
                        warmup(1)
                    continue
                if step[0] == "P":
                    it = step[1]
                    for bt in range(B_TILES):
                        mm_pre(it, bt)
                    continue
                bt, h, i0, i1 = step
                if (bt, h) == (2, 1):
                    for it in range(IT):
                        mm_sg(2, it, 0, SPL21, ptX21)
                    stage2_sg(2, 0, SPL21, ptX21)
                    for it in range(IT):
                        mm_sg(2, it, 1, SPL21, ptX21)
                    stage2_sg(2, 1, SPL21, ptX21)
                    continue
                if (bt, h) == (3, 1):
                    g31_opened = True
                    for it in range(IT):
                        mm31(it, sg=0)
                    stage2_31(0)
                    for it in range(IT):
                        mm31(it, sg=1)
                    stage2_31(1)
                    for it in range(IT):
                        mm31(it, sg=2)
                    stage2_31(2)
                    y_dma(1, 3)
                    continue
                for it in range(i0, i1):
                    mm(it, bt, h)
                if i1 == IT:
                    stage2(bt, h)
                    if bt % 2 == 1:
                        y_dma(h, bt)

    nc.compile()
    return nc


def _get_nc():
    if "nc" not in _nc_cache:
        _nc_cache["nc"] = _build_nc()
    return _nc_cache["nc"]


def _make_in_maps(x, W):
    import concourse.mybir as mybir

    mm_np = mybir.dt.np(getattr(mybir.dt, MM_DTYPE))
    x = np.asarray(x, dtype=np.float32)
    Wr = np.asarray(W, dtype=np.float32).reshape(D, D, C)
    # xT_dram[p, bt, it, q] = x[bt*P + q, it*IP + p]
    xTf = x.T.astype(mm_np)
    xT = np.ascontiguousarray(
        xTf.reshape(IT, IP, B_TILES, P).transpose(1, 2, 0, 3)
    )
    # xT2[p, it, b] = x[b, it*IP + p] for its 0-1 (it-major duplicate)
    xT2 = np.ascontiguousarray(xTf.reshape(IT, IP, B)[0:2].transpose(1, 0, 2))
    in_maps = []
    for c in range(NCORES):
        js, je = c * JS, (c + 1) * JS
        # wsh[i, k*JS + j] = Wr[i, js+j, k]; then [p, h, it, col] partition-major
        wsh = Wr[:, js:je, :].transpose(0, 2, 1).reshape(D, JK).astype(mm_np)
        wshard = np.ascontiguousarray(
            wsh.reshape(IT, IP, 2, HALF).transpose(1, 2, 0, 3)
        )
        # xs_dram[p, bt, j] = x[bt*P + p, js + j]
        xsl = np.ascontiguousarray(
            x[:, js:je].reshape(B_TILES, P, JS).transpose(1, 0, 2).astype(mm_np)
        )
        in_maps.append({"xT": xT, "xT2": xT2, "w": wshard, "xs": xsl})
    return in_maps


def run_spmd(x, W, **spmd_kwargs):
    """Compile/run the SPMD kernel; returns (partials, BassKernelResults)."""
    from concourse.bass_utils import run_bass_kernel_spmd

    nc = _get_nc()
    in_maps = _make_in_maps(x, W)
    res = run_bass_kernel_spmd(nc, in_maps, core_ids=list(range(NCORES)), **spmd_kwargs)
    # y_dram[p, h, bt, kh] -> y[bt*P + p, h*KH + kh]
    partials = [
        r["y"].transpose(2, 0, 1, 3).reshape(B_TILES, P, C).reshape(B, C)
        for r in res.results
    ]
    return partials, res


def kernel(x, W, b):
    partials, _ = run_spmd(x, W)
    y = np.sum(np.stack(partials, 0), axis=0, dtype=np.float64) + np.asarray(
        b, dtype=np.float64
    )
    return y.astype(np.float32)


# revision 26
# speedup vs baseline: 1.0200x; 1.0200x over previous
"""Trainium2 kernel for the bilinear form y[b,k] = sum_ij x[b,i] x[b,j] W[i,j,k] + b[k].

Shapes: x (512, 784) f32, W (614656=784*784, 10) f32, b (10,) f32 -> y (512, 10) f32.

Strategy (8 NeuronCores):
  - Shard the j axis of W.reshape(784, 784, 10) across cores: 98 j's per core.
    Each core reads W/8 + full x (~2.5 MB in fp16); compute is the long pole.
  - Stage 1 (TensorE): U[b, (k,j)] = sum_i x[b,i] * W[i, j_shard, k], x^T tiles
    stationary, W shard moving, accumulating over 7 uniform 112-row i-tiles
    into 8 PSUM banks (4 batch tiles x 2 column halves = 5 k's x 98 j's).
  - Stage 2 (DVE): U * xs multiply (PSUM read) + reduce over j, ~1.14us per
    group; groups must close >=1.2us apart or the DVE backs up.
  - Host: y = sum_c y_part_c + b  (20 KB per core; no collectives needed).

Schedule (calibrated against HW traces):
  - First input chunk completes ~5.6us after kernel start (1.6us framework
    entry + 0.7us queue op + ~3.3us ring latency); total input delivery runs
    at ~0.22-0.3 MB/us; y DMA completion costs a fixed ~1.9us after issue;
    framework teardown is a fixed ~8.5us.  The PE runs at half clock until
    the HAM boost, ~6.3us of GAP-FREE PE activity after the first matmul;
    any PE idle gap resets the ramp (costing ~5us) -- so dummy warmup
    matmuls bridge from t~2.1 until real data lands.
  - xT is laid out bt-major so each batch-tile's stationary set (0.2 MB)
    arrives independently: the first group closes at ~9.5us instead of ~14.
  - Matmul order chases delivery group-major ((bt,h) columns of work), so
    the 8 PSUM groups close every ~1.4us and all but the last DVE stage-2
    hide under the matmul stream.
  - y leaves in 4 group-pair DMAs on the gpsimd ring as pairs complete.
"""

import numpy as np

D = 784
B = 512
C = 10
NCORES = 8
JS = D // NCORES  # 98 j's per core
JK = JS * C  # 980 free columns per core, laid out as (k, j)
HALF = JK // 2  # 490 = 5 k's x 98 j's -> one PSUM bank
KH = C // 2  # 5 k's per half
P = 128
B_TILES = B // P  # 4
IT = 7  # i-tiles
IP = D // IT  # 112 rows per i-tile (uniform, no padding)
N_WARMUP_MM = 12  # dummy matmuls bridging until the first chunks land

MM_DTYPE = "float16"  # dtype of the matmul operands (and their DMA)

_nc_cache = {}


def _build_nc():
    import concourse.bacc as bacc
    import concourse.mybir as mybir
    import concourse.tile as tile

    mm_dt = getattr(mybir.dt, MM_DTYPE)
    f32 = mybir.dt.float32

    nc = bacc.Bacc("TRN2", target_bir_lowering=False)

    # Partition-major DRAM layouts (see _make_in_maps).
    xT = nc.dram_tensor("xT", [IP, B_TILES, IT, P], mm_dt, kind="ExternalInput")
    xT2 = nc.dram_tensor("xT2", [IP, 2, B], mm_dt, kind="ExternalInput")
    w = nc.dram_tensor("w", [IP, 2, IT, HALF], mm_dt, kind="ExternalInput")
    xs = nc.dram_tensor("xs", [P, B_TILES, JS], mm_dt, kind="ExternalInput")
    y = nc.dram_tensor("y", [P, 2, B_TILES, KH], f32, kind="ExternalOutput")

    with tile.TileContext(nc) as tc:
        with (
            tc.tile_pool(name="wpool", bufs=8) as wpool,
            tc.tile_pool(name="xpool", bufs=4) as xpool,
            tc.tile_pool(name="xspool", bufs=1) as xspool,
            tc.tile_pool(name="ypool", bufs=1) as ypool,
            tc.tile_pool(name="scratch", bufs=4) as spool,
            tc.tile_pool(name="psum", bufs=8, space="PSUM") as psum_pool,
        ):
            # Dummy warmup operands; memset on gpsimd right after entry so
            # the first warmup matmul (and the HAM ramp clock) starts ASAP.
            dmy_t = spool.tile([1, P], mm_dt, name="dmy_t", tag="dmy_t", bufs=1)
            dmy_s = spool.tile([IP, P], mm_dt, name="dmy_s", tag="dmy_s", bufs=1)
            dmy_m = spool.tile([IP, HALF], mm_dt, name="dmy_m", tag="dmy_m", bufs=1)
            nc.gpsimd.memset(dmy_t[:], 0.0)
            nc.gpsimd.memset(dmy_s[:], 0.0)
            nc.gpsimd.memset(dmy_m[:], 0.0)

            w_sb = {}  # (it, h) -> [IP, HALF] view

            def w_dma(eng, h, c0, c1):
                wt = wpool.tile(
                    [IP, c1 - c0, HALF], mm_dt, name=f"w_h{h}c{c0}", tag=f"wh{h}"
                )
                eng.dma_start(wt[:], w[:, h, c0:c1, :])
                for it in range(c0, c1):
                    w_sb[(it, h)] = wt[:, it - c0, :]

            xT_sb = {}  # bt -> [IP, IT, P]

            def xt_dma(bt):
                xt = xpool.tile([IP, IT, P], mm_dt, name=f"xt_b{bt}", tag="xt")
                nc.scalar.dma_start(xt[:], xT[:, bt])
                xT_sb[bt] = xt

            # Issue order == per-ring delivery order (queue ops ~650ns each).
            # sync: wh0[0:2], wh0[2:4], wh0[4:7], wh1[0:2], wh1[4:7]
            # scalar: xt-bt0..3, wh1[2:4]
            # gpsimd: xs, then the y pairs (ring stays warm)
            w_dma(nc.sync, 0, 0, 2)
            xt2_sb = xpool.tile([IP, 2, B], mm_dt, name="xt2", tag="xt2", bufs=1)
            nc.scalar.dma_start(xt2_sb[:], xT2[:])
            xt_dma(0)
            xt_dma(1)
            w_dma(nc.sync, 0, 2, 4)
            w_dma(nc.sync, 0, 4, 6)
            w_dma(nc.sync, 0, 6, 7)
            xt_dma(2)
            xt_dma(3)
            w_dma(nc.sync, 1, 0, 2)
            w_dma(nc.scalar, 1, 2, 4)
            w_dma(nc.sync, 1, 4, 7)
            xs_sb = xspool.tile([P, B_TILES, JS], mm_dt)
            nc.gpsimd.dma_start(xs_sb[:], xs[:])

            # PSUM: 8 accumulation groups (bt, h), one bank each.
            pts = {}
            for h in range(2):
                for bt in range(B_TILES):
                    pts[(bt, h)] = psum_pool.tile(
                        [P, HALF], f32, name=f"pt_b{bt}h{h}", tag="pt", bufs=8
                    )

            def warmup(n, bank=(3, 1)):
                # dummy matmuls into a not-yet-opened group's bank; each one
                # opens and closes its own accumulation group.
                for _ in range(n):
                    nc.tensor.matmul(
                        pts[bank][:], dmy_s[:], dmy_m[:], start=True, stop=True
                    )

            # First warmups use the [1,128] micro-dummy (memset ~160ns, LDW
            # ~10ns) so the PE (and the HAM ramp clock) starts ~0.7us earlier
            # than waiting for the full dummy memsets.
            for _ in range(3):
                nc.tensor.matmul(
                    pts[(3, 1)][:, :P], dmy_t[:], dmy_t[:], start=True, stop=True
                )
            warmup(N_WARMUP_MM)

            y_t = ypool.tile([P, 2, B_TILES, KH], f32)

            def mm_pre(it, bt):
                # prefix matmuls: stationary from the duplicated it-major
                # xT2 slice (lands with the first W chunk), h0 groups only
                nc.tensor.matmul(
                    pts[(bt, 0)][:],
                    xt2_sb[:, it, bt * P : (bt + 1) * P],
                    w_sb[(it, 0)][:],
                    start=(it == 0),
                    stop=False,
                )

            def mm(it, bt, h):
                nc.tensor.matmul(
                    pts[(bt, h)][:],
                    xT_sb[bt][:, it, :],
                    w_sb[(it, h)][:],
                    start=(it == 0),
                    stop=(it == IT - 1),
                )

            def stage2(bt, h):
                # Multiply on DVE (PSUM read), then reduce over j on DVE.
                pt = pts[(bt, h)]
                scr = spool.tile(
                    [P, HALF], f32, name=f"scr{bt}{h}", tag="scr", bufs=2
                )
                s3 = scr[:].rearrange("p (kh j) -> p kh j", kh=KH)
                p3 = pt[:].rearrange("p (kh j) -> p kh j", kh=KH)
                xs3 = xs_sb[:, bt, None, :].broadcast_to([P, KH, JS])
                nc.vector.tensor_tensor(s3, p3, xs3, mybir.AluOpType.mult)
                nc.vector.tensor_reduce(
                    out=y_t[:, h, bt, :],
                    in_=s3,
                    op=mybir.AluOpType.add,
                    axis=mybir.AxisListType.X,
                )

            def y_dma(h, bt):
                # ship groups (bt-1, h) and (bt, h) together
                nc.gpsimd.dma_start(
                    y[:, h, bt - 1 : bt + 1, :], y_t[:, h, bt - 1 : bt + 1, :]
                )

            # Group-chasing stream: each step is a run of i-tiles for one
            # (bt, h) group, ordered so every run's W/xT chunks have landed
            # and the 8 closures spread ~1.4us apart.
            # "W" entries are single warmup fillers (into g31's still-unopened
            # bank) absorbing DMA arrival jitter without a PE gap/ramp reset.
            # Column splits for the last-closing groups: sub-group
            # accumulators live in a slice of the group's own bank plus banks
            # recycled from early-closed groups (pool rotation: ptR1->g00,
            # ptR2->g10, ptR3->g20 banks), so only a small stage-2 trails
            # the last matmul.
            SPL31 = [(0, 2), (2, 2), (4, 1)]
            SPL21 = [(0, 2), (2, 3)]
            ptX31 = {0: pts[(3, 1)]}
            ptX31[1] = psum_pool.tile([P, 2 * JS], f32, name="ptR1", tag="pt", bufs=8)
            ptX31[2] = psum_pool.tile([P, JS], f32, name="ptR2", tag="pt", bufs=8)
            ptX21 = {0: pts[(2, 1)]}
            ptX21[1] = psum_pool.tile([P, 3 * JS], f32, name="ptR3", tag="pt", bufs=8)

            def mm_sg(bt, it, sg, splits, ptx):
                k0, nk = splits[sg]
                out = ptx[sg][:, : nk * JS]
                wsl = w_sb[(it, 1)][:, k0 * JS : (k0 + nk) * JS]
                nc.tensor.matmul(
                    out, xT_sb[bt][:, it, :], wsl,
                    start=(it == 0), stop=(it == IT - 1),
                )

            def stage2_sg(bt, sg, splits, ptx):
                k0, nk = splits[sg]
                pt = ptx[sg][:, : nk * JS]
                scr = spool.tile(
                    [P, nk * JS], f32, name=f"scr{bt}s{sg}", tag="scr", bufs=2
                )
                s3 = scr[:].rearrange("p (kh j) -> p kh j", kh=nk)
                p3 = pt.rearrange("p (kh j) -> p kh j", kh=nk)
                xs3 = xs_sb[:, bt, None, :].broadcast_to([P, nk, JS])
                nc.vector.tensor_tensor(s3, p3, xs3, mybir.AluOpType.mult)
                nc.vector.tensor_reduce(
                    out=y_t[:, 1, bt, k0 : k0 + nk],
                    in_=s3,
                    op=mybir.AluOpType.add,
                    axis=mybir.AxisListType.X,
                )

            def mm31(it, sg):
                mm_sg(3, it, sg, SPL31, ptX31)

            def stage2_31(sg):
                stage2_sg(3, sg, SPL31, ptX31)

            SCHED = [
                ("P", 0), ("P", 1),           # its0-1 h0 x 4bt via xT2
                "W",
                (0, 0, 2, 4), "W",
                (0, 0, 4, 7),                 # g00 close c1
                (1, 0, 2, 7),                 # g10 close c2
                "W",
                (2, 0, 2, 7),                 # g20 close c3
                (3, 0, 2, 7),                 # g30 close c4
                (0, 1, 0, 7),                 # g01 close c5
                (1, 1, 0, 7),                 # g11 close c6
                (2, 1, 0, 7),                 # g21 split close
                (3, 1, 0, 7),                 # g31 split close
            ]
            g31_opened = False
            for step in SCHED:
                if step == "W":
                    if not g31_opened:
                        warmup(1)
                    continue
                if step[0] == "P":
                    it = step[1]
                    for bt in range(B_TILES):
                        mm_pre(it, bt)
                    continue
                bt, h, i0, i1 = step
                if (bt, h) == (2, 1):
                    for it in range(IT):
                        mm_sg(2, it, 0, SPL21, ptX21)
                    stage2_sg(2, 0, SPL21, ptX21)
                    for it in range(IT):
                        mm_sg(2, it, 1, SPL21, ptX21)
                    stage2_sg(2, 1, SPL21, ptX21)
                    continue
                if (bt, h) == (3, 1):
                    g31_opened = True
                    for it in range(IT):
                        mm31(it, sg=0)
                    stage2_31(0)
                    for it in range(IT):
                        mm31(it, sg=1)
                    stage2_31(1)
                    for it in range(IT):
                        mm31(it, sg=2)
                    stage2_31(2)
                    y_dma(1, 3)
                    continue
                for it in range(i0, i1):
                    mm(it, bt, h)
                if i1 == IT:
                    stage2(bt, h)
                    if bt % 2 == 1:
                        y_dma(h, bt)

    nc.compile()
    return nc


def _get_nc():
    if "nc" not in _nc_cache:
        _nc_cache["nc"] = _build_nc()
    return _nc_cache["nc"]


def _make_in_maps(x, W):
    import concourse.mybir as mybir

    mm_np = mybir.dt.np(getattr(mybir.dt, MM_DTYPE))
    x = np.asarray(x, dtype=np.float32)
    Wr = np.asarray(W, dtype=np.float32).reshape(D, D, C)
    # xT_dram[p, bt, it, q] = x[bt*P + q, it*IP + p]
    xTf = x.T.astype(mm_np)
    xT = np.ascontiguousarray(
        xTf.reshape(IT, IP, B_TILES, P).transpose(1, 2, 0, 3)
    )
    # xT2[p, it, b] = x[b, it*IP + p] for its 0-1 (it-major duplicate)
    xT2 = np.ascontiguousarray(xTf.reshape(IT, IP, B)[0:2].transpose(1, 0, 2))
    in_maps = []
    for c in range(NCORES):
        js, je = c * JS, (c + 1) * JS
        # wsh[i, k*JS + j] = Wr[i, js+j, k]; then [p, h, it, col] partition-major
        wsh = Wr[:, js:je, :].transpose(0, 2, 1).reshape(D, JK).astype(mm_np)
        wshard = np.ascontiguousarray(
            wsh.reshape(IT, IP, 2, HALF).transpose(1, 2, 0, 3)
        )
        # xs_dram[p, bt, j] = x[bt*P + p, js + j]
        xsl = np.ascontiguousarray(
            x[:, js:je].reshape(B_TILES, P, JS).transpose(1, 0, 2).astype(mm_np)
        )
        in_maps.append({"xT": xT, "xT2": xT2, "w": wshard, "xs": xsl})
    return in_maps


def run_spmd(x, W, **spmd_kwargs):
    """Compile/run the SPMD kernel; returns (partials, BassKernelResults)."""
    from concourse.bass_utils import run_bass_kernel_spmd

    nc = _get_nc()
    in_maps = _make_in_maps(x, W)
    res = run_bass_kernel_spmd(nc, in_maps, core_ids=list(range(NCORES)), **spmd_kwargs)
    # y_dram[p, h, bt, kh] -> y[bt*P + p, h*KH + kh]
    partials = [
        r["y"].transpose(2, 0, 1, 3).reshape(B_TILES, P, C).reshape(B, C)
        for r in res.results
    ]
    return partials, res


def kernel(x, W, b):
    partials, _ = run_spmd(x, W)
    y = np.sum(np.stack(partials, 0), axis=0, dtype=np.float64) + np.asarray(
        b, dtype=np.float64
    )
    return y.astype(np.float32)
